# revision 18
# baseline (speedup 1.0000x reference)
"""BehaviorMoE Trainium2 kernel (8 NeuronCores, SPMD data-parallel over sorted tokens).

Contract: kernel(**inputs) takes FULL inputs as returned by setup_inputs() and
returns the FULL [8192, 1024] float32 output.

Strategy (v4):
  - Host: sort tokens by behavior id. Each behavior t in 1..4 owns two cores;
    each core gets M=896 tokens (7x128 tiles) of that single behavior, padded
    with masked b==0 filler. Leftover b==0 tokens (out = x + beta) never touch
    the device and are assembled on host.
  - Device (identical SPMD program, per-core data):
      Expert matmuls in bf16 (x and W; ~0.2% rel err); gate logits in fp32r
      off a separate f32 xT (softmax amplifies logit rounding, so bf16 is
      not usable there). All weights resident in SBUF. All input DMAs are
      issued from Sync in arrival-priority order, k-chunked so the opening
      e0/c0 wave streams as the data lands; late inputs (xtok, e2/e3
      weights) are issued behind an SBUF->SBUF dependency DMA on the gates
      so they don't steal HBM bandwidth from the critical path.
      Combine chain per (tile, half): e0 gate-scale (tensor_scalar -> PSUM
      banks recycle as soon as gates land), e1 STT, gate-combined bias
      (exps^T @ b_all as a bf16 PE matmul; PSUM -> SBUF via Scalar copy;
      joined by a GpSimd STT scaled with rm = mask/expsum), e2/e3 STTs
      (ping-pong bf16 SBUF accumulators; in-place DVE ops fault).
      e2+e3 run fused per tile so each tile's LayerNorm (bn_stats halves),
      normalize (ACT halves), residual (GpSimd) and output DMA hide under
      the next tile's matmul stream; the final tile splits its residual
      across GpSimd+Vector to shorten the exposed tail.
  - Host: scatter per-core outputs back to original token order.
"""

import os
import sys

import numpy as np
import ml_dtypes

for _p in ("/opt/trn_rl_repo", "/root/.axon_site/_ro/trn_rl_repo"):
    if os.path.isdir(_p) and _p not in sys.path:
        sys.path.append(_p)

from contextlib import ExitStack

from concourse import bacc, bass, masks, mybir, tile
from concourse.bass_utils import run_bass_kernel_spmd

F32 = mybir.dt.float32
F32R = mybir.dt.float32r
BF16 = mybir.dt.bfloat16
AX = mybir.AxisListType
ALU = mybir.AluOpType
ACTF = mybir.ActivationFunctionType

D = 1024            # model dim
N = 8192            # tokens
NB = 4              # behaviors
NESH = 3            # shared experts
NE = 4              # experts per behavior (3 shared + 1 specific)
EPS = 1e-5
NCORES = 8
M = 896             # tokens per core (7 tiles of 128)
KT = D // 128       # k tiles (contraction)
IT = M // 128       # token tiles per core
FH = 512            # feature half-tile (psum bank width in f32)
C1 = M - FH         # second logit token chunk (384)


def _build_program(trivial_affine: bool) -> bass.Bass:
    nc = bacc.Bacc()

    xt_d = nc.declare_dram_parameter("xt", [128, KT * M], F32R, isOutput=False)
    xtb_d = nc.declare_dram_parameter("xtb", [128, KT * M], BF16, isOutput=False)
    xtok_d = nc.declare_dram_parameter("xtok", [M, D], BF16, isOutput=False)
    wt_d = nc.declare_dram_parameter("wt", [NE, 2, 128, KT * FH], BF16, isOutput=False)
    wg_d = nc.declare_dram_parameter("wg", [128, KT * NE], F32R, isOutput=False)
    ball_d = nc.declare_dram_parameter("ball", [NE, D], BF16, isOutput=False)
    mask_d = nc.declare_dram_parameter("mask", [128, IT], F32, isOutput=False)
    if not trivial_affine:
        gam_d = nc.declare_dram_parameter("gam", [128, D], F32, isOutput=False)
        bet_d = nc.declare_dram_parameter("bet", [128, D], F32, isOutput=False)
    out_d = nc.declare_dram_parameter("out", [M, D], F32, isOutput=True)

    with tile.TileContext(nc) as tc, ExitStack() as ctx:
        const = ctx.enter_context(tc.tile_pool(name="const", bufs=1))
        wpool = ctx.enter_context(tc.tile_pool(name="w", bufs=2 * NE))
        selp = ctx.enter_context(tc.tile_pool(name="sel", bufs=2 * IT))
        xtokp = ctx.enter_context(tc.tile_pool(name="xtok", bufs=IT))
        outp = ctx.enter_context(tc.tile_pool(name="outp", bufs=3))
        scrp = ctx.enter_context(tc.tile_pool(name="scr", bufs=3))
        biasp = ctx.enter_context(tc.tile_pool(name="bias", bufs=3))
        gatep = ctx.enter_context(tc.tile_pool(name="gate", bufs=IT))
        gtsp = ctx.enter_context(tc.tile_pool(name="gts", bufs=2 * IT))
        smallp = ctx.enter_context(tc.tile_pool(name="small", bufs=16))
        zpool = ctx.enter_context(tc.tile_pool(name="z", bufs=6, space="PSUM"))
        pspool = ctx.enter_context(tc.tile_pool(name="ps", bufs=2, space="PSUM"))

        # ---- DMA issues, all on Sync in arrival-priority order ----
        wg_sb = const.tile([128, KT * NE], F32R, tag="wg")
        nc.sync.dma_start(wg_sb[:], wg_d[:])
        xtb = const.tile([128, KT * M], BF16, tag="xtb")
        w_sb = {}
        for e in range(NE):
            for c in (0, 1):
                w_sb[(e, c)] = wpool.tile(
                    [128, KT * FH], BF16, tag="w", name=f"w{e}{c}"
                )
        # opening wave data, k-chunked and interleaved
        for j in range(KT):
            xs = slice(j * M, (j + 1) * M)
            ws = slice(j * FH, (j + 1) * FH)
            nc.sync.dma_start(xtb[:, xs], xtb_d[:, xs])
            nc.sync.dma_start(w_sb[(0, 0)][:, ws], wt_d[0, 0, :, ws])
        xT = const.tile([128, KT * M], F32R, tag="xt")
        half = KT * M // 2
        nc.sync.dma_start(xT[:, 0:half], xt_d[:, 0:half])
        nc.sync.dma_start(xT[:, half:], xt_d[:, half:])
        mask_sb = const.tile([128, IT], F32, tag="mask")
        nc.sync.dma_start(mask_sb[:], mask_d[:])
        ball_sb = const.tile([NE, D], BF16, tag="ball")
        nc.sync.dma_start(ball_sb[:], ball_d[:])
        nc.sync.dma_start(w_sb[(0, 1)][:], wt_d[0, 1])
        nc.sync.dma_start(w_sb[(1, 0)][:], wt_d[1, 0])
        nc.sync.dma_start(w_sb[(1, 1)][:], wt_d[1, 1])
        if not trivial_affine:
            gam_sb = const.tile([128, D], F32, tag="gam")
            nc.sync.dma_start(gam_sb[:], gam_d[:])
            bet_sb = const.tile([128, D], F32, tag="bet")
            nc.sync.dma_start(bet_sb[:], bet_d[:])

        identity = const.tile([128, 128], F32, tag="ident")
        masks.make_identity(nc, identity[:])
        identB = const.tile([128, 128], BF16, tag="identB")
        nc.vector.tensor_copy(identB[:], identity[:])

        # ---- accumulators (ping-pong; in-place DVE ops fault) ----
        selA = [selp.tile([128, D], BF16, tag="sel", name=f"selA{i}")
                for i in range(IT)]
        selB = [selp.tile([128, D], BF16, tag="sel", name=f"selB{i}")
                for i in range(IT)]

        def isl(i):
            return slice(i * 128, (i + 1) * 128)

        def csl(c):
            return slice(c * FH, (c + 1) * FH)

        def expert_mms(zt, e, c, i):
            for k in range(KT):
                nc.tensor.matmul(
                    zt[:], xtb[:, k * M + i * 128:k * M + (i + 1) * 128],
                    w_sb[(e, c)][:, k * FH:(k + 1) * FH],
                    start=(k == 0), stop=(k == KT - 1),
                )

        def expert_group(e, c, i):
            zt = zpool.tile([128, FH], F32, tag="z")
            expert_mms(zt, e, c, i)
            return zt

        # ---- e0/c0 wave for tiles 0..5, k-outer (paced by the k-chunk DMAs) ----
        zt_e0c0 = [zpool.tile([128, FH], F32, tag="z", name=f"zw{i}")
                   for i in range(6)]
        for k in range(KT):
            for i in range(6):
                nc.tensor.matmul(
                    zt_e0c0[i][:], xtb[:, k * M + isl(i).start:k * M + isl(i).stop],
                    w_sb[(0, 0)][:, k * FH:(k + 1) * FH],
                    start=(k == 0), stop=(k == KT - 1),
                )

        # ---- gate logits (fp32r), transpose, masked softmax ----
        glc0 = pspool.tile([128, FH], F32, tag="ps", name="glc0")
        glc1 = pspool.tile([128, FH], F32, tag="ps", name="glc1")
        for k in range(KT):
            nc.tensor.matmul(
                glc0[0:NE, :], wg_sb[:, k * NE:(k + 1) * NE],
                xT[:, k * M:k * M + FH],
                start=(k == 0), stop=(k == KT - 1),
            )
            nc.tensor.matmul(
                glc1[0:NE, 0:C1], wg_sb[:, k * NE:(k + 1) * NE],
                xT[:, k * M + FH:(k + 1) * M],
                start=(k == 0), stop=(k == KT - 1),
            )
        glT_sb = const.tile([NE, M], F32R, tag="glT")
        nc.vector.tensor_copy(glT_sb[:, 0:FH], glc0[0:NE, :])
        nc.vector.tensor_copy(glT_sb[:, FH:M], glc1[0:NE, 0:C1])
        identR = const.tile([NE, NE], F32R, tag="identR")
        nc.vector.tensor_copy(identR[:], identity[0:NE, 0:NE])

        # all 7 logit transposes into ONE psum bank (disjoint column ranges,
        # concurrent tiny MM groups) so one pspool slot is freed for tile 6's
        # e0/c0 group and the softmax never gates the PE on pool slots
        glp_all = pspool.tile([128, FH], F32, tag="ps", name="glp")
        for i in range(IT):
            nc.tensor.matmul(
                glp_all[:, NE * i:NE * (i + 1)], glT_sb[:, isl(i)], identR[:],
                start=True, stop=True, skip_group_check=True,
            )
        zt6 = pspool.tile([128, FH], F32, tag="ps", name="zt6")
        expert_mms(zt6, 0, 0, 6)

        # software-pipelined masked softmax: Vector never waits on Scalar's
        # exp, and each tile's e0/c0 combine (-> PSUM bank free) follows its
        # gates immediately
        gates_t = []
        exps_t = []
        rm_t = []
        negmax_t = []
        expsum_t = []
        for i in range(IT):
            negmax = smallp.tile([128, 1], F32, tag="s1")
            nc.vector.tensor_reduce(
                negmax[:], glp_all[:, NE * i:NE * (i + 1)],
                axis=AX.X, op=ALU.max, negate=True,
            )
            negmax_t.append(negmax)
            exps = smallp.tile([128, NE], F32, tag="s4")
            expsum = smallp.tile([128, 1], F32, tag="s1")
            nc.scalar.activation(
                exps[:], glp_all[:, NE * i:NE * (i + 1)], ACTF.Exp,
                bias=negmax[:], scale=1.0, accum_out=expsum[:],
            )
            exps_t.append(exps)
            expsum_t.append(expsum)
        for i in range(IT):
            rinv = smallp.tile([128, 1], F32, tag="s1")
            nc.vector.reciprocal(rinv[:], expsum_t[i][:])
            rm = smallp.tile([128, 1], F32, tag="rm", name=f"rm{i}")
            nc.vector.tensor_mul(rm[:], rinv[:], mask_sb[:, i:i + 1])
            rm_t.append(rm)
            gates = gatep.tile([128, NE], F32, tag="g")
            nc.vector.tensor_scalar_mul(gates[:], exps_t[i][:], rm[:])
            gates_t.append(gates)
            zt = zt_e0c0[i] if i < 6 else zt6
            nc.vector.tensor_scalar_mul(
                selB[i][:, csl(0)], zt[:], gates[:, 0:1]
            )

        # late inputs: gate their issue on gates_t[0] so they don't steal
        # HBM bandwidth from the opening critical path
        depgate = const.tile([128, NE], F32, tag="depg")
        nc.sync.dma_start(depgate[:], gates_t[0][:])
        xtok_t = []
        for i in range(IT):
            xi = xtokp.tile([128, D], BF16, tag="xtok", name=f"xi{i}")
            nc.sync.dma_start(xi[:], xtok_d[isl(i), :])
            xtok_t.append(xi)
        for e in (2, 3):
            for c in (0, 1):
                nc.sync.dma_start(w_sb[(e, c)][:], wt_d[e, c])

        # ---- e0/c1 ----
        for i in range(IT):
            zt = expert_group(0, 1, i)
            nc.vector.tensor_scalar_mul(
                selB[i][:, csl(1)], zt[:], gates_t[i][:, 0:1]
            )

        # exps^T via PE (bf16) for the bias combine
        expsT_t = []
        for i in range(IT):
            expsB = gtsp.tile([128, NE], BF16, tag="eB", name=f"eB{i}")
            nc.vector.tensor_copy(expsB[:], exps_t[i][:])
            gtp = pspool.tile([128, FH], F32, tag="ps", name=f"gtp{i}")
            nc.tensor.matmul(
                gtp[0:NE, 0:128], expsB[:], identB[:], start=True, stop=True
            )
            expsT = gtsp.tile([NE, 128], BF16, tag="eT", name=f"eT{i}")
            nc.vector.tensor_copy(expsT[:], gtp[0:NE, 0:128])
            expsT_t.append(expsT)

        # ---- e1 (+ bias combine join per (c, tile)) ----
        for c in (0, 1):
            for i in range(IT):
                zt = expert_group(1, c, i)
                nc.vector.scalar_tensor_tensor(
                    selA[i][:, csl(c)], zt[:], gates_t[i][:, 1:2],
                    selB[i][:, csl(c)], op0=ALU.mult, op1=ALU.add,
                )
                bp = pspool.tile([128, FH], F32, tag="ps", name=f"bp{i}{c}")
                nc.tensor.matmul(
                    bp[:], expsT_t[i][:], ball_sb[:, csl(c)],
                    start=True, stop=True,
                )
                bias_sb = biasp.tile([128, FH], F32, tag="bias")
                nc.scalar.copy(bias_sb[:], bp[:])
                nc.vector.scalar_tensor_tensor(
                    selB[i][:, csl(c)], bias_sb[:], rm_t[i][:],
                    selA[i][:, csl(c)], op0=ALU.mult, op1=ALU.add,
                )

        # ---- fused e2+e3 passes with per-tile LN tail ----
        # Each tile's post-bn LN chain is deferred until after the NEXT
        # tile's STTs so the Vector stream frees PSUM banks back-to-back.
        def ln_tail(i, bn6):
            selF = selB[i]
            mv = smallp.tile([128, 2], F32, tag="mv")
            nc.vector.bn_aggr(mv[:], bn6[:])
            avi = smallp.tile([128, 1], F32, tag="s1")
            nc.vector.tensor_scalar_add(avi[:], mv[:, 1:2], EPS)
            sdi = smallp.tile([128, 1], F32, tag="s1")
            nc.scalar.sqrt(sdi[:], avi[:])
            ri = smallp.tile([128, 1], F32, tag="s1")
            nc.vector.reciprocal(ri[:], sdi[:])
            mbi = smallp.tile([128, 1], F32, tag="s1")
            nc.vector.scalar_tensor_tensor(
                mbi[:], mv[:, 0:1], -1.0, ri[:], op0=ALU.mult, op1=ALU.mult
            )
            # ln = sel*rstd + mb on ACT (halves), residual on GpSimd
            # (last tile: split across GpSimd+Vector to shorten the tail)
            xi = xtok_t[i]
            outt = outp.tile([128, D], F32, tag="out")
            for c in (0, 1):
                lnb = scrp.tile([128, FH], F32, tag="scr")
                nc.scalar.activation(
                    lnb[:], selF[:, csl(c)], ACTF.Identity,
                    bias=mbi[:], scale=ri[:],
                )
                if not trivial_affine:
                    lng = scrp.tile([128, FH], F32, tag="scr")
                    nc.vector.tensor_mul(lng[:], lnb[:], gam_sb[:, csl(c)])
                    lnb2 = scrp.tile([128, FH], F32, tag="scr")
                    nc.vector.tensor_add(lnb2[:], lng[:], bet_sb[:, csl(c)])
                    lnb = lnb2
                eng = nc.vector if (c == 1 and i == IT - 1) else nc.gpsimd
                eng.tensor_add(outt[:, csl(c)], lnb[:], xi[:, csl(c)])
                nc.sync.dma_start(out_d[isl(i), csl(c)], outt[:, csl(c)])

        pending = None
        for i in range(IT):
            bn6 = smallp.tile([128, 2 * 6], F32, tag="bn6")
            for c in (0, 1):
                zt2 = expert_group(2, c, i)
                nc.vector.scalar_tensor_tensor(
                    selA[i][:, csl(c)], zt2[:], gates_t[i][:, 2:3],
                    selB[i][:, csl(c)], op0=ALU.mult, op1=ALU.add,
                )
                zt3 = expert_group(3, c, i)
                nc.vector.scalar_tensor_tensor(
                    selB[i][:, csl(c)], zt3[:], gates_t[i][:, 3:4],
                    selA[i][:, csl(c)], op0=ALU.mult, op1=ALU.add,
                )
                nc.vector.bn_stats(bn6[:, 6 * c:6 * c + 6], selB[i][:, csl(c)])
                if pending is not None and c == 0:
                    ln_tail(*pending)
                    pending = None
            pending = (i, bn6)
        ln_tail(*pending)

    nc.finalize()
    return nc


_PROGRAM_CACHE: dict = {}


def _get_program(trivial_affine: bool) -> bass.Bass:
    key = trivial_affine
    if key not in _PROGRAM_CACHE:
        _PROGRAM_CACHE[key] = _build_program(trivial_affine)
    return _PROGRAM_CACHE[key]


def _pack_tokens(b: np.ndarray):
    """Two cores per behavior t in 1..4, M=896 tokens each, padded with masked
    b==0 filler. Returns (per-core (idx, mask, t) list, leftover b==0 idx)."""
    idx0 = np.flatnonzero(b == 0)
    p0 = 0
    cores = []
    for t in range(1, NB + 1):
        idxs = np.flatnonzero(b == t)
        if len(idxs) > 2 * M:
            raise RuntimeError(
                f"behavior {t} has {len(idxs)} tokens > capacity {2 * M}"
            )
        for s in (0, M):
            part = idxs[s:s + M]
            need = M - len(part)
            fill = idx0[p0:p0 + need]
            p0 += need
            if len(fill) != need:
                raise RuntimeError("not enough b==0 filler tokens for packing")
            idx = np.concatenate([part.astype(np.int64), fill.astype(np.int64)])
            msk = np.zeros((M,), np.float32)
            msk[:len(part)] = 1.0
            cores.append((idx, msk, t))
    return cores, idx0[p0:]


def _behavior_tensors(W_sh, b_sh, W_sp, b_sp, w_gates):
    per_t = {}
    W_sh_flat = W_sh.reshape(NESH * D, D)
    for t in range(1, NB + 1):
        Wall = np.concatenate([W_sh_flat, W_sp[t - 1:t].reshape(D, D)], axis=0)
        wT = np.ascontiguousarray(Wall.T)                      # [D, NE*D]
        # [e, c, p, k*FH + f] = wT[128k + p, e*D + c*FH + f]
        wt_h = np.ascontiguousarray(
            wT.reshape(KT, 128, NE, 2, FH).transpose(2, 3, 1, 0, 4)
            .reshape(NE, 2, 128, KT * FH).astype(ml_dtypes.bfloat16)
        )
        # [p, k*NE + e] = w_gates[t-1][128k + p, e]
        wg_h = np.ascontiguousarray(
            w_gates[t - 1].reshape(KT, 128, NE).transpose(1, 0, 2)
            .reshape(128, KT * NE)
        )
        ball_h = np.stack([b_sh[0], b_sh[1], b_sh[2], b_sp[t - 1]], axis=0)
        per_t[t] = (wt_h, wg_h,
                    np.ascontiguousarray(ball_h).astype(ml_dtypes.bfloat16))
    return per_t


def _prepare(x, b_seq, W_sh, b_sh, W_sp, b_sp, w_gates, gamma, beta):
    x = np.ascontiguousarray(np.asarray(x, dtype=np.float32))
    b = np.asarray(b_seq).astype(np.int64).ravel()
    W_sh = np.asarray(W_sh, dtype=np.float32)
    b_sh = np.asarray(b_sh, dtype=np.float32)
    W_sp = np.asarray(W_sp, dtype=np.float32)
    b_sp = np.asarray(b_sp, dtype=np.float32)
    w_gates = np.asarray(w_gates, dtype=np.float32)
    gamma = np.asarray(gamma, dtype=np.float32)
    beta = np.asarray(beta, dtype=np.float32)
    assert x.shape == (N, D) and b.shape == (N,)

    trivial = bool(np.all(gamma == 1.0) and np.all(beta == 0.0))
    cores, leftover = _pack_tokens(b)
    per_t = _behavior_tensors(W_sh, b_sh, W_sp, b_sp, w_gates)

    in_maps = []
    for idx, msk, t in cores:
        wt_h, wg_h, ball_h = per_t[t]
        xc = np.ascontiguousarray(x[idx])                      # [M, D]
        # [p, k*M + m] = x[m, 128k + p]
        xt_h = np.ascontiguousarray(
            xc.T.reshape(KT, 128, M).transpose(1, 0, 2).reshape(128, KT * M)
        )
        m = {
            "xt": xt_h,
            "xtb": xt_h.astype(ml_dtypes.bfloat16),
            "xtok": xc.astype(ml_dtypes.bfloat16),
            "wt": wt_h,
            "wg": wg_h,
            "ball": ball_h,
            "mask": np.ascontiguousarray(msk.reshape(IT, 128).T),
        }
        if not trivial:
            m["gam"] = np.ascontiguousarray(np.broadcast_to(gamma, (128, D)))
            m["bet"] = np.ascontiguousarray(np.broadcast_to(beta, (128, D)))
        in_maps.append(m)
    return trivial, cores, leftover, in_maps, x, beta


def kernel_with_results(trace: bool = False, **inputs):
    trivial, cores, leftover, in_maps, x, beta = _prepare(**inputs)
    nc = _get_program(trivial)
    res = run_bass_kernel_spmd(
        nc, in_maps, list(range(NCORES)), trace=trace
    )
    out = np.empty((N, D), np.float32)
    for c, (idx, _msk, _t) in enumerate(cores):
        out[idx] = res.results[c]["out"]
    if len(leftover):
        out[leftover] = x[leftover] + beta[None, :]
    return out, res


def kernel(**inputs) -> np.ndarray:
    out, _ = kernel_with_results(trace=False, **inputs)
    return out


# revision 20
# speedup vs baseline: 1.0008x; 1.0008x over previous
"""BehaviorMoE Trainium2 kernel (8 NeuronCores, SPMD data-parallel over sorted tokens).

Contract: kernel(**inputs) takes FULL inputs as returned by setup_inputs() and
returns the FULL [8192, 1024] float32 output.

Strategy (v4):
  - Host: sort tokens by behavior id. Each behavior t in 1..4 owns two cores;
    each core gets M=896 tokens (7x128 tiles) of that single behavior, padded
    with masked b==0 filler. Leftover b==0 tokens (out = x + beta) never touch
    the device and are assembled on host.
  - Device (identical SPMD program, per-core data):
      Expert matmuls in bf16 (x and W; ~0.2% rel err); gate logits in fp32r
      off a separate f32 xT (softmax amplifies logit rounding, so bf16 is
      not usable there). All weights resident in SBUF. All input DMAs are
      issued from Sync in arrival-priority order, k-chunked so the opening
      e0/c0 wave streams as the data lands; late inputs (xtok, e2/e3
      weights) are issued behind an SBUF->SBUF dependency DMA on the gates
      so they don't steal HBM bandwidth from the critical path.
      Combine chain per (tile, half): e0 gate-scale (tensor_scalar -> PSUM
      banks recycle as soon as gates land), e1 STT, gate-combined bias
      (exps^T @ b_all as a bf16 PE matmul; PSUM -> SBUF via Scalar copy;
      joined by a GpSimd STT scaled with rm = mask/expsum), e2/e3 STTs
      (ping-pong bf16 SBUF accumulators; in-place DVE ops fault).
      e2+e3 run fused per tile so each tile's LayerNorm (bn_stats halves),
      normalize (ACT halves), residual (GpSimd) and output DMA hide under
      the next tile's matmul stream; the final tile splits its residual
      across GpSimd+Vector to shorten the exposed tail.
  - Host: scatter per-core outputs back to original token order.
"""

import os
import sys

import numpy as np
import ml_dtypes

for _p in ("/opt/trn_rl_repo", "/root/.axon_site/_ro/trn_rl_repo"):
    if os.path.isdir(_p) and _p not in sys.path:
        sys.path.append(_p)

from contextlib import ExitStack

from concourse import bacc, bass, masks, mybir, tile
from concourse.bass_utils import run_bass_kernel_spmd

F32 = mybir.dt.float32
F32R = mybir.dt.float32r
BF16 = mybir.dt.bfloat16
AX = mybir.AxisListType
ALU = mybir.AluOpType
ACTF = mybir.ActivationFunctionType

D = 1024            # model dim
N = 8192            # tokens
NB = 4              # behaviors
NESH = 3            # shared experts
NE = 4              # experts per behavior (3 shared + 1 specific)
EPS = 1e-5
NCORES = 8
M = 896             # tokens per core (7 tiles of 128)
KT = D // 128       # k tiles (contraction)
IT = M // 128       # token tiles per core
FH = 512            # feature half-tile (psum bank width in f32)
C1 = M - FH         # second logit token chunk (384)


def _build_program(trivial_affine: bool) -> bass.Bass:
    nc = bacc.Bacc()

    xt_d = nc.declare_dram_parameter("xt", [128, KT * M], F32R, isOutput=False)
    xtb_d = nc.declare_dram_parameter("xtb", [128, KT * M], BF16, isOutput=False)
    xtok_d = nc.declare_dram_parameter("xtok", [M, D], BF16, isOutput=False)
    wt_d = nc.declare_dram_parameter("wt", [NE, 2, 128, KT * FH], BF16, isOutput=False)
    wg_d = nc.declare_dram_parameter("wg", [128, KT * NE], F32R, isOutput=False)
    ball_d = nc.declare_dram_parameter("ball", [NE, D], BF16, isOutput=False)
    mask_d = nc.declare_dram_parameter("mask", [128, IT], F32, isOutput=False)
    if not trivial_affine:
        gam_d = nc.declare_dram_parameter("gam", [128, D], F32, isOutput=False)
        bet_d = nc.declare_dram_parameter("bet", [128, D], F32, isOutput=False)
    out_d = nc.declare_dram_parameter("out", [M, D], F32, isOutput=True)

    with tile.TileContext(nc) as tc, ExitStack() as ctx:
        const = ctx.enter_context(tc.tile_pool(name="const", bufs=1))
        wpool = ctx.enter_context(tc.tile_pool(name="w", bufs=2 * NE))
        selp = ctx.enter_context(tc.tile_pool(name="sel", bufs=2 * IT))
        xtokp = ctx.enter_context(tc.tile_pool(name="xtok", bufs=IT))
        outp = ctx.enter_context(tc.tile_pool(name="outp", bufs=3))
        scrp = ctx.enter_context(tc.tile_pool(name="scr", bufs=3))
        biasp = ctx.enter_context(tc.tile_pool(name="bias", bufs=3))
        gatep = ctx.enter_context(tc.tile_pool(name="gate", bufs=IT))
        gtsp = ctx.enter_context(tc.tile_pool(name="gts", bufs=2 * IT))
        smallp = ctx.enter_context(tc.tile_pool(name="small", bufs=16))
        zpool = ctx.enter_context(tc.tile_pool(name="z", bufs=6, space="PSUM"))
        pspool = ctx.enter_context(tc.tile_pool(name="ps", bufs=2, space="PSUM"))

        # ---- DMA issues, all on Sync in arrival-priority order ----
        wg_sb = const.tile([128, KT * NE], F32R, tag="wg")
        nc.sync.dma_start(wg_sb[:], wg_d[:])
        xtb = const.tile([128, KT * M], BF16, tag="xtb")
        w_sb = {}
        for e in range(NE):
            for c in (0, 1):
                w_sb[(e, c)] = wpool.tile(
                    [128, KT * FH], BF16, tag="w", name=f"w{e}{c}"
                )
        # opening wave data, k-chunked and interleaved
        for j in range(KT):
            xs = slice(j * M, (j + 1) * M)
            ws = slice(j * FH, (j + 1) * FH)
            nc.sync.dma_start(xtb[:, xs], xtb_d[:, xs])
            nc.sync.dma_start(w_sb[(0, 0)][:, ws], wt_d[0, 0, :, ws])
        xT = const.tile([128, KT * M], F32R, tag="xt")
        half = KT * M // 2
        nc.scalar.dma_start(xT[:, 0:half], xt_d[:, 0:half])
        nc.scalar.dma_start(xT[:, half:], xt_d[:, half:])
        mask_sb = const.tile([128, IT], F32, tag="mask")
        nc.scalar.dma_start(mask_sb[:], mask_d[:])
        ball_sb = const.tile([NE, D], BF16, tag="ball")
        nc.scalar.dma_start(ball_sb[:], ball_d[:])
        nc.sync.dma_start(w_sb[(0, 1)][:], wt_d[0, 1])
        nc.sync.dma_start(w_sb[(1, 0)][:], wt_d[1, 0])
        nc.sync.dma_start(w_sb[(1, 1)][:], wt_d[1, 1])
        if not trivial_affine:
            gam_sb = const.tile([128, D], F32, tag="gam")
            nc.sync.dma_start(gam_sb[:], gam_d[:])
            bet_sb = const.tile([128, D], F32, tag="bet")
            nc.sync.dma_start(bet_sb[:], bet_d[:])

        identity = const.tile([128, 128], F32, tag="ident")
        masks.make_identity(nc, identity[:])
        identB = const.tile([128, 128], BF16, tag="identB")
        nc.vector.tensor_copy(identB[:], identity[:])

        # ---- accumulators (ping-pong; in-place DVE ops fault) ----
        selA = [selp.tile([128, D], BF16, tag="sel", name=f"selA{i}")
                for i in range(IT)]
        selB = [selp.tile([128, D], BF16, tag="sel", name=f"selB{i}")
                for i in range(IT)]

        def isl(i):
            return slice(i * 128, (i + 1) * 128)

        def csl(c):
            return slice(c * FH, (c + 1) * FH)

        def expert_mms(zt, e, c, i):
            for k in range(KT):
                nc.tensor.matmul(
                    zt[:], xtb[:, k * M + i * 128:k * M + (i + 1) * 128],
                    w_sb[(e, c)][:, k * FH:(k + 1) * FH],
                    start=(k == 0), stop=(k == KT - 1),
                )

        def expert_group(e, c, i):
            zt = zpool.tile([128, FH], F32, tag="z")
            expert_mms(zt, e, c, i)
            return zt

        # ---- e0/c0 wave for tiles 0..5, k-outer (paced by the k-chunk DMAs) ----
        zt_e0c0 = [zpool.tile([128, FH], F32, tag="z", name=f"zw{i}")
                   for i in range(6)]
        for k in range(KT):
            for i in range(6):
                nc.tensor.matmul(
                    zt_e0c0[i][:], xtb[:, k * M + isl(i).start:k * M + isl(i).stop],
                    w_sb[(0, 0)][:, k * FH:(k + 1) * FH],
                    start=(k == 0), stop=(k == KT - 1),
                )

        # ---- gate logits (fp32r), transpose, masked softmax ----
        glc0 = pspool.tile([128, FH], F32, tag="ps", name="glc0")
        glc1 = pspool.tile([128, FH], F32, tag="ps", name="glc1")
        for k in range(KT):
            nc.tensor.matmul(
                glc0[0:NE, :], wg_sb[:, k * NE:(k + 1) * NE],
                xT[:, k * M:k * M + FH],
                start=(k == 0), stop=(k == KT - 1),
            )
            nc.tensor.matmul(
                glc1[0:NE, 0:C1], wg_sb[:, k * NE:(k + 1) * NE],
                xT[:, k * M + FH:(k + 1) * M],
                start=(k == 0), stop=(k == KT - 1),
            )
        glT_sb = const.tile([NE, M], F32R, tag="glT")
        nc.vector.tensor_copy(glT_sb[:, 0:FH], glc0[0:NE, :])
        nc.vector.tensor_copy(glT_sb[:, FH:M], glc1[0:NE, 0:C1])
        identR = const.tile([NE, NE], F32R, tag="identR")
        nc.vector.tensor_copy(identR[:], identity[0:NE, 0:NE])

        # all 7 logit transposes into ONE psum bank (disjoint column ranges,
        # concurrent tiny MM groups) so one pspool slot is freed for tile 6's
        # e0/c0 group and the softmax never gates the PE on pool slots
        glp_all = pspool.tile([128, FH], F32, tag="ps", name="glp")
        for i in range(IT):
            nc.tensor.matmul(
                glp_all[:, NE * i:NE * (i + 1)], glT_sb[:, isl(i)], identR[:],
                start=True, stop=True, skip_group_check=True,
            )
        zt6 = pspool.tile([128, FH], F32, tag="ps", name="zt6")
        expert_mms(zt6, 0, 0, 6)

        # software-pipelined masked softmax: Vector never waits on Scalar's
        # exp, and each tile's e0/c0 combine (-> PSUM bank free) follows its
        # gates immediately
        gates_t = []
        exps_t = []
        rm_t = []
        negmax_t = []
        expsum_t = []
        for i in range(IT):
            negmax = smallp.tile([128, 16], F32, tag="s1", name="negmax")[:, 0:1]
            nc.vector.tensor_reduce(
                negmax[:], glp_all[:, NE * i:NE * (i + 1)],
                axis=AX.X, op=ALU.max, negate=True,
            )
            negmax_t.append(negmax)
            exps = smallp.tile([128, 16], F32, tag="s4", name="exps")[:, 0:NE]
            expsum = smallp.tile([128, 16], F32, tag="s1", name="expsum")[:, 0:1]
            nc.scalar.activation(
                exps[:], glp_all[:, NE * i:NE * (i + 1)], ACTF.Exp,
                bias=negmax[:], scale=1.0, accum_out=expsum[:],
            )
            exps_t.append(exps)
            expsum_t.append(expsum)
        for i in range(IT):
            rinv = smallp.tile([128, 16], F32, tag="s1", name="rinv")[:, 0:1]
            nc.vector.reciprocal(rinv[:], expsum_t[i][:])
            rm = smallp.tile([128, 16], F32, tag="rm", name=f"rm{i}")[:, 0:1]
            nc.vector.tensor_mul(rm[:], rinv[:], mask_sb[:, i:i + 1])
            rm_t.append(rm)
            gates = gatep.tile([128, 16], F32, tag="g", name="gates")[:, 0:NE]
            nc.vector.tensor_scalar_mul(gates[:], exps_t[i][:], rm[:])
            gates_t.append(gates)
            zt = zt_e0c0[i] if i < 6 else zt6
            nc.vector.tensor_scalar_mul(
                selB[i][:, csl(0)], zt[:], gates[:, 0:1]
            )

        # late inputs: gate their issue on gates_t[0] so they don't steal
        # HBM bandwidth from the opening critical path
        depgate = const.tile([128, NE], F32, tag="depg")
        nc.sync.dma_start(depgate[:], gates_t[0][:])
        xtok_t = []
        for i in range(IT):
            xi = xtokp.tile([128, D], BF16, tag="xtok", name=f"xi{i}")
            nc.sync.dma_start(xi[:], xtok_d[isl(i), :])
            xtok_t.append(xi)
        for e in (2, 3):
            for c in (0, 1):
                nc.sync.dma_start(w_sb[(e, c)][:], wt_d[e, c])

        # ---- e0/c1 ----
        for i in range(IT):
            zt = expert_group(0, 1, i)
            nc.vector.tensor_scalar_mul(
                selB[i][:, csl(1)], zt[:], gates_t[i][:, 0:1]
            )

        # exps^T via PE (bf16) for the bias combine
        expsT_t = []
        for i in range(IT):
            expsB = gtsp.tile([128, 32], BF16, tag="eB", name=f"eB{i}")[:, 0:NE]
            nc.vector.tensor_copy(expsB[:], exps_t[i][:])
            gtp = pspool.tile([128, FH], F32, tag="ps", name=f"gtp{i}")
            nc.tensor.matmul(
                gtp[0:NE, 0:128], expsB[:], identB[:], start=True, stop=True
            )
            expsT = gtsp.tile([NE, 128], BF16, tag="eT", name=f"eT{i}")
            nc.vector.tensor_copy(expsT[:], gtp[0:NE, 0:128])
            expsT_t.append(expsT)

        # ---- e1 (+ bias combine join per (c, tile)) ----
        for c in (0, 1):
            for i in range(IT):
                zt = expert_group(1, c, i)
                nc.vector.scalar_tensor_tensor(
                    selA[i][:, csl(c)], zt[:], gates_t[i][:, 1:2],
                    selB[i][:, csl(c)], op0=ALU.mult, op1=ALU.add,
                )
                bp = pspool.tile([128, FH], F32, tag="ps", name=f"bp{i}{c}")
                nc.tensor.matmul(
                    bp[:], expsT_t[i][:], ball_sb[:, csl(c)],
                    start=True, stop=True,
                )
                bias_sb = biasp.tile([128, FH], F32, tag="bias")
                nc.scalar.copy(bias_sb[:], bp[:])
                nc.vector.scalar_tensor_tensor(
                    selB[i][:, csl(c)], bias_sb[:], rm_t[i][:],
                    selA[i][:, csl(c)], op0=ALU.mult, op1=ALU.add,
                )

        # ---- fused e2+e3 passes with per-tile LN tail ----
        # Each tile's post-bn LN chain is deferred until after the NEXT
        # tile's STTs so the Vector stream frees PSUM banks back-to-back.
        def ln_tail(i, bn6):
            selF = selB[i]
            mv = smallp.tile([128, 16], F32, tag="mv", name="mv")[:, 0:2]
            nc.vector.bn_aggr(mv[:], bn6[:])
            avi = smallp.tile([128, 16], F32, tag="s1", name="avi")[:, 0:1]
            nc.vector.tensor_scalar_add(avi[:], mv[:, 1:2], EPS)
            sdi = smallp.tile([128, 16], F32, tag="s1", name="sdi")[:, 0:1]
            nc.scalar.sqrt(sdi[:], avi[:])
            ri = smallp.tile([128, 16], F32, tag="s1", name="ri")[:, 0:1]
            nc.vector.reciprocal(ri[:], sdi[:])
            mbi = smallp.tile([128, 16], F32, tag="s1", name="mbi")[:, 0:1]
            nc.vector.scalar_tensor_tensor(
                mbi[:], mv[:, 0:1], -1.0, ri[:], op0=ALU.mult, op1=ALU.mult
            )
            # ln = sel*rstd + mb on ACT (halves), residual on GpSimd
            # (last tile: split across GpSimd+Vector to shorten the tail)
            xi = xtok_t[i]
            outt = outp.tile([128, D], F32, tag="out")
            for c in (0, 1):
                lnb = scrp.tile([128, FH], F32, tag="scr")
                nc.scalar.activation(
                    lnb[:], selF[:, csl(c)], ACTF.Identity,
                    bias=mbi[:], scale=ri[:],
                )
                if not trivial_affine:
                    lng = scrp.tile([128, FH], F32, tag="scr")
                    nc.vector.tensor_mul(lng[:], lnb[:], gam_sb[:, csl(c)])
                    lnb2 = scrp.tile([128, FH], F32, tag="scr")
                    nc.vector.tensor_add(lnb2[:], lng[:], bet_sb[:, csl(c)])
                    lnb = lnb2
                eng = nc.vector if (c == 1 and i == IT - 1) else nc.gpsimd
                eng.tensor_add(outt[:, csl(c)], lnb[:], xi[:, csl(c)])
                nc.sync.dma_start(out_d[isl(i), csl(c)], outt[:, csl(c)])

        pending = None
        for i in range(IT):
            bn6 = smallp.tile([128, 16], F32, tag="bn6", name="bn6")[:, 0:12]
            for c in (0, 1):
                zt2 = expert_group(2, c, i)
                nc.vector.scalar_tensor_tensor(
                    selA[i][:, csl(c)], zt2[:], gates_t[i][:, 2:3],
                    selB[i][:, csl(c)], op0=ALU.mult, op1=ALU.add,
                )
                zt3 = expert_group(3, c, i)
                nc.vector.scalar_tensor_tensor(
                    selB[i][:, csl(c)], zt3[:], gates_t[i][:, 3:4],
                    selA[i][:, csl(c)], op0=ALU.mult, op1=ALU.add,
                )
                nc.vector.bn_stats(bn6[:, 6 * c:6 * c + 6], selB[i][:, csl(c)])
                if pending is not None and c == 0:
                    ln_tail(*pending)
                    pending = None
            pending = (i, bn6)
        ln_tail(*pending)

    nc.finalize()
    return nc


_PROGRAM_CACHE: dict = {}


def _get_program(trivial_affine: bool) -> bass.Bass:
    key = trivial_affine
    if key not in _PROGRAM_CACHE:
        _PROGRAM_CACHE[key] = _build_program(trivial_affine)
    return _PROGRAM_CACHE[key]


def _pack_tokens(b: np.ndarray):
    """Two cores per behavior t in 1..4, M=896 tokens each, padded with masked
    b==0 filler. Returns (per-core (idx, mask, t) list, leftover b==0 idx)."""
    idx0 = np.flatnonzero(b == 0)
    p0 = 0
    cores = []
    for t in range(1, NB + 1):
        idxs = np.flatnonzero(b == t)
        if len(idxs) > 2 * M:
            raise RuntimeError(
                f"behavior {t} has {len(idxs)} tokens > capacity {2 * M}"
            )
        for s in (0, M):
            part = idxs[s:s + M]
            need = M - len(part)
            fill = idx0[p0:p0 + need]
            p0 += need
            if len(fill) != need:
                raise RuntimeError("not enough b==0 filler tokens for packing")
            idx = np.concatenate([part.astype(np.int64), fill.astype(np.int64)])
            msk = np.zeros((M,), np.float32)
            msk[:len(part)] = 1.0
            cores.append((idx, msk, t))
    return cores, idx0[p0:]


def _behavior_tensors(W_sh, b_sh, W_sp, b_sp, w_gates):
    per_t = {}
    W_sh_flat = W_sh.reshape(NESH * D, D)
    for t in range(1, NB + 1):
        Wall = np.concatenate([W_sh_flat, W_sp[t - 1:t].reshape(D, D)], axis=0)
        wT = np.ascontiguousarray(Wall.T)                      # [D, NE*D]
        # [e, c, p, k*FH + f] = wT[128k + p, e*D + c*FH + f]
        wt_h = np.ascontiguousarray(
            wT.reshape(KT, 128, NE, 2, FH).transpose(2, 3, 1, 0, 4)
            .reshape(NE, 2, 128, KT * FH).astype(ml_dtypes.bfloat16)
        )
        # [p, k*NE + e] = w_gates[t-1][128k + p, e]
        wg_h = np.ascontiguousarray(
            w_gates[t - 1].reshape(KT, 128, NE).transpose(1, 0, 2)
            .reshape(128, KT * NE)
        )
        ball_h = np.stack([b_sh[0], b_sh[1], b_sh[2], b_sp[t - 1]], axis=0)
        per_t[t] = (wt_h, wg_h,
                    np.ascontiguousarray(ball_h).astype(ml_dtypes.bfloat16))
    return per_t


def _prepare(x, b_seq, W_sh, b_sh, W_sp, b_sp, w_gates, gamma, beta):
    x = np.ascontiguousarray(np.asarray(x, dtype=np.float32))
    b = np.asarray(b_seq).astype(np.int64).ravel()
    W_sh = np.asarray(W_sh, dtype=np.float32)
    b_sh = np.asarray(b_sh, dtype=np.float32)
    W_sp = np.asarray(W_sp, dtype=np.float32)
    b_sp = np.asarray(b_sp, dtype=np.float32)
    w_gates = np.asarray(w_gates, dtype=np.float32)
    gamma = np.asarray(gamma, dtype=np.float32)
    beta = np.asarray(beta, dtype=np.float32)
    assert x.shape == (N, D) and b.shape == (N,)

    trivial = bool(np.all(gamma == 1.0) and np.all(beta == 0.0))
    cores, leftover = _pack_tokens(b)
    per_t = _behavior_tensors(W_sh, b_sh, W_sp, b_sp, w_gates)

    in_maps = []
    for idx, msk, t in cores:
        wt_h, wg_h, ball_h = per_t[t]
        xc = np.ascontiguousarray(x[idx])                      # [M, D]
        # [p, k*M + m] = x[m, 128k + p]
        xt_h = np.ascontiguousarray(
            xc.T.reshape(KT, 128, M).transpose(1, 0, 2).reshape(128, KT * M)
        )
        m = {
            "xt": xt_h,
            "xtb": xt_h.astype(ml_dtypes.bfloat16),
            "xtok": xc.astype(ml_dtypes.bfloat16),
            "wt": wt_h,
            "wg": wg_h,
            "ball": ball_h,
            "mask": np.ascontiguousarray(msk.reshape(IT, 128).T),
        }
        if not trivial:
            m["gam"] = np.ascontiguousarray(np.broadcast_to(gamma, (128, D)))
            m["bet"] = np.ascontiguousarray(np.broadcast_to(beta, (128, D)))
        in_maps.append(m)
    return trivial, cores, leftover, in_maps, x, beta


def kernel_with_results(trace: bool = False, **inputs):
    trivial, cores, leftover, in_maps, x, beta = _prepare(**inputs)
    nc = _get_program(trivial)
    res = run_bass_kernel_spmd(
        nc, in_maps, list(range(NCORES)), trace=trace
    )
    out = np.empty((N, D), np.float32)
    for c, (idx, _msk, _t) in enumerate(cores):
        out[idx] = res.results[c]["out"]
    if len(leftover):
        out[leftover] = x[leftover] + beta[None, :]
    return out, res


def kernel(**inputs) -> np.ndarray:
    out, _ = kernel_with_results(trace=False, **inputs)
    return out
